# revision 1
# baseline (speedup 1.0000x reference)
"""Causal self-attention Trainium2 kernel (8 NeuronCores, SPMD).

Sharding: 8 cores = 2 batches x 4 head-groups (4 heads of 64 dims each).
Each core computes full-sequence attention for its 4 heads plus the
partial output projection for its 256 y-columns; the host sums the 4
partials per batch and adds the output bias.

Layout strategy (no on-device transposes anywhere):
  - host supplies x[b].T as xT [C, T] (bf16)
  - qT, kT produced in [m, t] layout (W stationary, xT moving)
  - v produced in natural [t, m] layout (xT stationary, Wv moving),
    augmented with a ones column per head (M=65) so the attention-value
    matmul also emits the softmax denominator row for free
  - attT[j, i] = sum_d kT[d,j] qT[d,i]  (kT stationary K=64; two heads
    run concurrently via row-tiled tile_position)
  - exp on ScalarE (fused 1/sqrt(64) scale), causal crossing blocks
    masked with precomputed multiplicative mask tiles
  - yT_aug[65, i] accumulated over j-tiles; row 64 = softmax sum
  - per-ib lazy normalization: batched reciprocal + PE selector-matmul\n    broadcast of 1/s, cast bf16; out-projection interleaved per i-block
  - out[t, n] = yT.T @ Wp partial, f32, DMA'd out
"""

import sys

for _p in ("/opt/trn_rl_repo",):
    if _p not in sys.path:
        sys.path.insert(0, _p)

from contextlib import ExitStack

import ml_dtypes
import numpy as np

import concourse.bass as bass
import concourse.tile as tile
from concourse import bacc, mybir
from concourse.bass_utils import run_bass_kernel_spmd

BF16 = mybir.dt.bfloat16
F32 = mybir.dt.float32
NP_BF16 = ml_dtypes.bfloat16

B, T, C = 2, 2048, 1024
H, D = 16, 64
N_CORES = 8
GROUPS = 4          # head groups (cores per batch)
MH = C // GROUPS    # 256 columns per core (4 heads)
LH = MH // D        # 4 local heads
CT = C // 128       # 8 contraction tiles
TT = T // 128       # 16 sequence tiles of 128
IB = T // 512       # 4 i-blocks of 512
SCALE = 1.0 / np.sqrt(D)


def _selector() -> np.ndarray:
    """sel[k, r*64+j] = 1.0 if k == r else 0, [8, 512] bf16 — K=8 matmul
    broadcasts row r of an [8, 512] tile to 64 output partitions."""
    sel = np.zeros((8, 512), dtype=NP_BF16)
    for r in range(8):
        sel[r, r * 64 : (r + 1) * 64] = 1.0
    return sel


def _causal_masks() -> np.ndarray:
    """mask[r][j, i] = 1.0 if j + 128*r <= i else 0 (bf16), [4, 128, 512]."""
    r = np.arange(4)[:, None, None]
    j = np.arange(128)[None, :, None]
    i = np.arange(512)[None, None, :]
    return (j + 128 * r <= i).astype(NP_BF16)


def emit_kernel(
    nc, xT_d, wq_d, wk_d, wv_d, wp_d, bq_d, bk_d, bv_d, out_d, masks_d, sel_d
):
    with tile.TileContext(nc) as tc, ExitStack() as ctx:
        # ---- long-lived tiles -------------------------------------------
        keep = ctx.enter_context(tc.tile_pool(name="keep", bufs=1))
        qT_s = keep.tile([128, 2, T], BF16, tag="qT")
        kT_s = keep.tile([128, 2, T], BF16, tag="kT")
        v_s = keep.tile([128, TT, LH, D + 1], BF16, tag="v")
        yTn_s = keep.tile([128, 2, T], BF16, tag="yTn")
        wp_s = keep.tile([128, 2, C], BF16, tag="wp")
        mask_st = keep.tile([128, 4, 512], BF16, tag="mask_st")
        mask_s = keep.tile([128, 4, 512], BF16, tag="mask")
        bq_st = keep.tile([128, 2], F32, tag="bq_st")
        bq_s = keep.tile([128, 2], F32, tag="bq")
        bk_st = keep.tile([128, 2], F32, tag="bk_st")
        bk_s = keep.tile([128, 2], F32, tag="bk")
        bv_row = keep.tile([1, MH], F32, tag="bv_row")
        bv_row_bf = keep.tile([1, MH], BF16, tag="bv_row_bf")
        bv_bc = keep.tile([128, MH], F32, tag="bv_bc")
        ones_bf128 = keep.tile([1, 128], BF16, tag="ones_bf128")
        ones_bf = keep.tile([1, 64], BF16, tag="ones_bf")
        sel_s = keep.tile([8, 512], BF16, tag="sel")

        nc.vector.memset(ones_bf128[:], 1.0)
        nc.vector.memset(ones_bf[:], 1.0)
        nc.vector.memset(v_s[:, :, :, D : D + 1], 1.0)

        # ---- phase 1: projections --------------------------------------
        with (
            tc.tile_pool(name="proj_in", bufs=1) as pin,
            tc.tile_pool(name="proj_ps", bufs=4, space="PSUM") as pps,
        ):
            xT_s = pin.tile([128, CT, T], BF16, tag="xT")
            wq_s = pin.tile([128, CT, MH], BF16, tag="wq")
            wk_s = pin.tile([128, CT, MH], BF16, tag="wk")
            wv_s = pin.tile([128, CT, MH], BF16, tag="wv")
            # SP issues DMAs in program order: first-needed data first
            xT_r = xT_d.ap().rearrange("(o p) t -> p o t", p=128)
            wq_r = wq_d.ap().rearrange("(o p) m -> p o m", p=128)
            wk_r = wk_d.ap().rearrange("(o p) m -> p o m", p=128)
            wv_r = wv_d.ap().rearrange("(o p) m -> p o m", p=128)

            def xt_chunk(tb):
                nc.sync.dma_start(
                    xT_s[:, :, tb * 512 : (tb + 1) * 512],
                    xT_r[:, :, tb * 512 : (tb + 1) * 512],
                )

            nc.sync.dma_start(wq_s[:], wq_r[:])
            xt_chunk(0)
            nc.sync.dma_start(wk_s[:], wk_r[:])
            xt_chunk(1)
            nc.sync.dma_start(wv_s[:], wv_r[:])
            xt_chunk(2)
            xt_chunk(3)
            wp_r = wp_d.ap().rearrange("(o p) n -> p o n", p=128)
            nc.sync.dma_start(wp_s[:], wp_r[:])
            # consts staged through a DVE copy: consumers then depend on DVE
            # program order instead of a DMA semaphore (walrus 1-wait limit)
            nc.gpsimd.dma_start(mask_st[:], masks_d.ap().rearrange("m p i -> p m i"))
            nc.gpsimd.dma_start(bq_st[:], bq_d.ap().rearrange("(o p) -> p o", p=128))
            nc.gpsimd.dma_start(bk_st[:], bk_d.ap().rearrange("(o p) -> p o", p=128))
            nc.gpsimd.dma_start(bv_row[:], bv_d.ap()[None, :])
            nc.gpsimd.dma_start(sel_s[:], sel_d.ap())
            nc.vector.tensor_copy(mask_s[:], mask_st[:])
            nc.vector.tensor_copy(bq_s[:], bq_st[:])
            nc.vector.tensor_copy(bk_s[:], bk_st[:])
            nc.vector.tensor_copy(bv_row_bf[:], bv_row[:])
            bv_ps = pps.tile([128, MH], F32, tag="v_ps", name="bv_ps")
            nc.tensor.matmul(
                bv_ps[:], ones_bf128[:], bv_row_bf[:], start=True, stop=True
            )
            nc.vector.tensor_copy(bv_bc[:], bv_ps[:])

            # qT / kT: [m, t] = W.T @ x.T  (W stationary)
            for w_s, b_s, dst in ((wq_s, bq_s, qT_s), (wk_s, bk_s, kT_s)):
                for mt in range(2):
                    for tb in range(IB):
                        ps = pps.tile([128, 512], F32, tag="proj_ps")
                        for ct in range(CT):
                            nc.tensor.matmul(
                                ps[:],
                                w_s[:, ct, mt * 128 : (mt + 1) * 128],
                                xT_s[:, ct, tb * 512 : (tb + 1) * 512],
                                start=(ct == 0),
                                stop=(ct == CT - 1),
                            )
                        nc.vector.tensor_scalar(
                            dst[:, mt, tb * 512 : (tb + 1) * 512],
                            ps[:],
                            b_s[:, mt : mt + 1],
                            None,
                            mybir.AluOpType.add,
                        )

            # v natural [t, m]  (xT stationary)
            for tt in range(TT):
                ps = pps.tile([128, MH], F32, tag="v_ps")
                for ct in range(CT):
                    nc.tensor.matmul(
                        ps[:],
                        xT_s[:, ct, tt * 128 : (tt + 1) * 128],
                        wv_s[:, ct, :],
                        start=(ct == 0),
                        stop=(ct == CT - 1),
                    )
                nc.vector.tensor_tensor(
                    v_s[:, tt, :, 0:D],
                    ps[:].rearrange("p (h d) -> p h d", h=LH),
                    bv_bc[:].rearrange("p (h d) -> p h d", h=LH),
                    mybir.AluOpType.add,
                )

        # ---- phase 2+3: attention per pair, out-proj interleaved ---------
        with (
            tc.tile_pool(name="pt", bufs=1) as ptp,
            tc.tile_pool(name="att_ps", bufs=2, space="PSUM") as aps,
            tc.tile_pool(name="yt_ps", bufs=2, space="PSUM") as yps,
            tc.tile_pool(name="s_ps", bufs=1, space="PSUM") as sps,
            tc.tile_pool(name="out_ps", bufs=1, space="PSUM") as ops,
            tc.tile_pool(name="norm", bufs=2) as npool,
            tc.tile_pool(name="out_sb", bufs=2) as osb,
        ):
            out_r = out_d.ap().rearrange("(tt p) n -> tt p n", p=128)
            for p in range(2):
                PT = [
                    ptp.tile([128, TT, T], BF16, tag=f"PT{lh}", name=f"PT{lh}")
                    for lh in range(2)
                ]
                yTu = npool.tile([64, 8, 512], BF16, tag="yTu", name="yTu")
                for ib in range(IB):
                    yT_ps = [
                        yps.tile([D + 1, 512], F32, tag="yT_ps", name=f"yT_ps{lh}")
                        for lh in range(2)
                    ]
                    for jt in range(4 * ib + 4):
                        if jt // 4 == ib:
                            # first use of this j-tile: attT + exp from the
                            # 128-aligned diagonal column ia onward only
                            ia = 128 * jt
                            w_all = T - ia
                            off = 0
                            while off < w_all:
                                cw = min(1024, w_all - off)
                                for lh in range(2):
                                    att_ps = aps.tile([128, 1024], F32, tag="att_ps")
                                    prow = slice(64 * lh, 64 * lh + 64)
                                    for s5 in range(0, cw, 512):
                                        nn = min(512, cw - s5)
                                        nc.tensor.matmul(
                                            att_ps[:, s5 : s5 + nn],
                                            kT_s[prow, p, jt * 128 : (jt + 1) * 128],
                                            qT_s[prow, p, ia + off + s5 : ia + off + s5 + nn],
                                            start=True,
                                            stop=True,
                                        )
                                    nc.scalar.activation(
                                        PT[lh][:, jt, ia + off : ia + off + cw],
                                        att_ps[:, :cw],
                                        mybir.ActivationFunctionType.Exp,
                                        scale=float(SCALE),
                                    )
                                    if off == 0:
                                        # diagonal 128x128 tile: zero j > i
                                        nc.vector.tensor_tensor(
                                            PT[lh][:, jt, ia : ia + 128],
                                            PT[lh][:, jt, ia : ia + 128],
                                            mask_s[:, 0, 0:128],
                                            mybir.AluOpType.mult,
                                        )
                                off += cw
                        for lh in range(2):
                            ia = 128 * jt
                            c0 = max(512 * ib, ia)
                            nc.tensor.matmul(
                                yT_ps[lh][:, c0 - 512 * ib : 512],
                                v_s[:, jt, 2 * p + lh, :],
                                PT[lh][:, jt, c0 : 512 * ib + 512],
                                start=(jt == 0),
                                stop=(jt == 4 * ib + 3),
                            )
                    # lazy per-ib normalization: stash yT + s, recip, scale.
                    srows = npool.tile([2, 512], F32, tag="srows", name="srows")
                    for lh in range(2):
                        r = ib * 2 + lh
                        nc.vector.tensor_copy(yTu[:, r, :], yT_ps[lh][0:D, :])
                        sstage = npool.tile([1, 512], F32, tag="sstage", name="sstage")
                        nc.vector.tensor_copy(sstage[:], yT_ps[lh][D : D + 1, :])
                        nc.sync.dma_start(srows[lh : lh + 1, :], sstage[:])
                    rs2 = npool.tile([2, 512], BF16, tag="rs2", name="rs2")
                    with nc.allow_low_precision(
                        reason="1/s broadcast via bf16 matmul; bf16 noise ~0.4% ok"
                    ):
                        nc.vector.reciprocal(rs2[:], srows[:])
                    for lh in range(2):
                        S_ps = sps.tile([64, 512], F32, tag="S_ps", name="S_ps")
                        nc.tensor.matmul(
                            S_ps[:],
                            sel_s[0:2, lh * 64 : (lh + 1) * 64],
                            rs2[:],
                            start=True,
                            stop=True,
                        )
                        nc.vector.tensor_tensor(
                            yTn_s[64 * lh : 64 * lh + 64, p, 512 * ib : 512 * ib + 512],
                            yTu[:, ib * 2 + lh, :],
                            S_ps[:],
                            mybir.AluOpType.mult,
                        )
                    if p == 1:
                        # both pairs normalized for this ib: project + store
                        for tt in range(4 * ib, 4 * ib + 4):
                            for nb in range(2):
                                o_ps = ops.tile(
                                    [128, 512], F32, tag="out_ps", name="o_ps"
                                )
                                for pp in range(2):
                                    nc.tensor.matmul(
                                        o_ps[:],
                                        yTn_s[:, pp, tt * 128 : (tt + 1) * 128],
                                        wp_s[:, pp, nb * 512 : (nb + 1) * 512],
                                        start=(pp == 0),
                                        stop=(pp == 1),
                                    )
                                ot = osb.tile([128, 512], F32, tag="out_t")
                                nc.vector.tensor_copy(ot[:], o_ps[:])
                                nc.sync.dma_start(
                                    out_r[tt, :, nb * 512 : (nb + 1) * 512], ot[:]
                                )


_NC_CACHE = None


def get_nc() -> bass.Bass:
    global _NC_CACHE
    if _NC_CACHE is None:
        nc = bacc.Bacc()
        xT_d = nc.declare_dram_parameter("xT", [C, T], BF16, isOutput=False)
        wq_d = nc.declare_dram_parameter("wq", [C, MH], BF16, isOutput=False)
        wk_d = nc.declare_dram_parameter("wk", [C, MH], BF16, isOutput=False)
        wv_d = nc.declare_dram_parameter("wv", [C, MH], BF16, isOutput=False)
        wp_d = nc.declare_dram_parameter("wp", [MH, C], BF16, isOutput=False)
        bq_d = nc.declare_dram_parameter("bq", [MH], F32, isOutput=False)
        bk_d = nc.declare_dram_parameter("bk", [MH], F32, isOutput=False)
        bv_d = nc.declare_dram_parameter("bv", [MH], F32, isOutput=False)
        out_d = nc.declare_dram_parameter("out", [T, C], F32, isOutput=True)
        masks_d = nc.inline_tensor(_causal_masks(), name="causal_masks")
        sel_d = nc.inline_tensor(_selector(), name="selector")
        emit_kernel(
            nc, xT_d, wq_d, wk_d, wv_d, wp_d, bq_d, bk_d, bv_d, out_d, masks_d, sel_d
        )
        nc.finalize()
        _NC_CACHE = nc
    return _NC_CACHE


def make_in_maps(x, Wq, bq, Wk, bk, Wv, bv, Wp, bp):
    in_maps = []
    for core in range(N_CORES):
        b, g = divmod(core, GROUPS)
        sl = slice(g * MH, (g + 1) * MH)
        in_maps.append(
            {
                "xT": np.ascontiguousarray(x[b].T).astype(NP_BF16),
                "wq": np.ascontiguousarray(Wq[:, sl]).astype(NP_BF16),
                "wk": np.ascontiguousarray(Wk[:, sl]).astype(NP_BF16),
                "wv": np.ascontiguousarray(Wv[:, sl]).astype(NP_BF16),
                "wp": np.ascontiguousarray(Wp[sl, :]).astype(NP_BF16),
                "bq": np.ascontiguousarray(bq[sl]).astype(np.float32),
                "bk": np.ascontiguousarray(bk[sl]).astype(np.float32),
                "bv": np.ascontiguousarray(bv[sl]).astype(np.float32),
            }
        )
    return in_maps


def kernel(x, Wq, bq, Wk, bk, Wv, bv, Wp, bp, _results_hook=None, _trace=False):
    x = np.asarray(x, dtype=np.float32)
    nc = get_nc()
    in_maps = make_in_maps(x, Wq, bq, Wk, bk, Wv, bv, Wp, bp)
    res = run_bass_kernel_spmd(
        nc, in_maps, core_ids=list(range(N_CORES)), trace=_trace
    )
    if _results_hook is not None:
        _results_hook(res)
    out = np.zeros((B, T, C), dtype=np.float32)
    for core in range(N_CORES):
        b = core // GROUPS
        out[b] += res.results[core]["out"]
    out += np.asarray(bp, dtype=np.float32)[None, None, :]
    return out



# revision 8
# speedup vs baseline: 1.2203x; 1.2203x over previous
"""Causal self-attention Trainium2 kernel (8 NeuronCores, SPMD).

Sharding: 8 cores = 2 batches x 4 head-groups (4 heads of 64 dims each).
Each core computes full-sequence attention for its 4 heads plus the
partial output projection for its 256 y-columns; the host sums the 4
partials per batch and adds the output bias.

Layout strategy (no on-device transposes anywhere):
  - host supplies x[b].T as xT [C, T] (bf16)
  - qT, kT produced in [m, t] layout (W stationary, xT moving)
  - v produced in natural [t, m] layout (xT stationary, Wv moving), with
    bias folded in as an extra K=1 ones-row matmul and a ones column per
    head (M=65) so the attention-value matmul also emits the softmax
    denominator row for free
  - attT[j, i] = sum_d kT[d,j] qT[d,i]  (kT stationary K=64; two heads
    run concurrently via row-tiled tile_position)
  - exp on ScalarE (fused 1/sqrt(64) scale); causal diagonal blocks
    masked multiplicatively on GpSimd; PT strips packed triangularly
    (strip jt keeps only columns >= 128*jt) so both head-pairs' strips
    fit in SBUF at once
  - software-pipelined emission: pair-1 q/k projections interleave with
    pair-0's first exp strips, v-projection with pair-1's; per i-block
    the PV accumulation interleaves with the next i-block's QK^T/exp
    and the previous i-block's output projection
  - normalization: denominator row staged to SBUF, reciprocal_approx_
    fast, K=1 ones-matmul broadcast, multiply
  - out[t, n] = yT.T @ Wp partial, bf16, DMA'd out; host sums in f32
"""

import sys

for _p in ("/opt/trn_rl_repo",):
    if _p not in sys.path:
        sys.path.insert(0, _p)

from contextlib import ExitStack

import ml_dtypes
import numpy as np

import concourse.bass as bass
import concourse.tile as tile
from concourse import bacc, mybir
from concourse.bass_utils import run_bass_kernel_spmd

BF16 = mybir.dt.bfloat16
F32 = mybir.dt.float32
NP_BF16 = ml_dtypes.bfloat16

B, T, C = 2, 2048, 1024
H, D = 16, 64
N_CORES = 8
GROUPS = 4          # head groups (cores per batch)
MH = C // GROUPS    # 256 columns per core (4 heads)
LH = MH // D        # 4 local heads
CT = C // 128       # 8 contraction tiles
TT = T // 128       # 16 sequence tiles of 128
IB = T // 512       # 4 i-blocks of 512
SCALE = 1.0 / np.sqrt(D)

# packed triangular PT layout: strip jt holds columns [128*jt, T)
OFF = []
_o = 0
for _jt in range(TT):
    OFF.append(_o)
    _o += T - 128 * _jt
PT_W = _o  # 17408


def _causal_mask() -> np.ndarray:
    """mask[j, i] = 1.0 if j <= i else 0 (bf16), [128, 128]."""
    j = np.arange(128)[:, None]
    i = np.arange(128)[None, :]
    return (j <= i).astype(NP_BF16)


def emit_kernel(nc, xT_d, wq_d, wk_d, wv_d, wp_d, bq_d, bk_d, bv_d, out_d, masks_d):
    with tile.TileContext(nc) as tc, ExitStack() as ctx:
        # ---- long-lived tiles -------------------------------------------
        keep = ctx.enter_context(tc.tile_pool(name="keep", bufs=1))
        qT_s = keep.tile([128, 2, T], BF16, tag="qT")
        kT_s = keep.tile([128, 2, T], BF16, tag="kT")
        v_s = keep.tile([128, TT, LH, D + 1], BF16, tag="v")
        mask_st = keep.tile([128, 128], BF16, tag="mask_st")
        mask_s = keep.tile([128, 128], BF16, tag="mask")
        bq_st = keep.tile([128, 2], F32, tag="bq_st")
        bq_s = keep.tile([128, 2], F32, tag="bq")
        bk_st = keep.tile([128, 2], F32, tag="bk_st")
        bk_s = keep.tile([128, 2], F32, tag="bk")
        bv_row = keep.tile([1, MH], F32, tag="bv_row")
        bv_row_bf = keep.tile([1, MH], BF16, tag="bv_row_bf")
        ones_bf128 = keep.tile([1, 128], BF16, tag="ones_bf128")
        ones_bf = keep.tile([1, 64], BF16, tag="ones_bf")
        act_scr = keep.tile([1, 128], F32, tag="act_scr")

        nc.vector.memset(ones_bf128[:], 1.0)
        nc.vector.memset(ones_bf[:], 1.0)
        nc.vector.memset(act_scr[:], 0.0)
        nc.vector.memset(v_s[:, :, :, D : D + 1], 1.0)
        # preload the exp table set while input DMAs are in flight
        nc.scalar.activation(
            act_scr[:], act_scr[:], mybir.ActivationFunctionType.Exp, scale=1.0
        )

        # ---- projections + pair-0/1 first strips, software-pipelined ----
        ptp = ctx.enter_context(tc.tile_pool(name="pt", bufs=1))
        aps = ctx.enter_context(tc.tile_pool(name="att_ps", bufs=2, space="PSUM"))
        pin_cm = tc.tile_pool(name="proj_in", bufs=1)
        pin = pin_cm.__enter__()
        pps_cm = tc.tile_pool(name="proj_ps", bufs=2, space="PSUM")
        pps = pps_cm.__enter__()
        PT = [
            [ptp.tile([128, PT_W], BF16, tag=f"PT{p}{lh}", name=f"PT{p}{lh}")
             for lh in range(2)]
            for p in range(2)
        ]

        xT_s = pin.tile([128, CT, T], BF16, tag="xT")
        wq_s = pin.tile([128, CT, MH], BF16, tag="wq")
        wk_s = pin.tile([128, CT, MH], BF16, tag="wk")
        wv_s = pin.tile([128, CT, MH], BF16, tag="wv")
        xT_r = xT_d.ap().rearrange("(o p) t -> p o t", p=128)
        wq_r = wq_d.ap().rearrange("(o p) m -> p o m", p=128)
        wk_r = wk_d.ap().rearrange("(o p) m -> p o m", p=128)
        wv_r = wv_d.ap().rearrange("(o p) m -> p o m", p=128)

        # warm the PE clock gate with dummy matmuls during the input DMAs
        for _ in range(32):
            wps = pps.tile([128, 512], F32, tag="proj_ps", name="warm_ps")
            nc.tensor.matmul(
                wps[:, 0:128], ones_bf128[:], ones_bf128[:], start=True, stop=True
            )

        # SP issues DMAs in program order: first-needed data first
        nc.sync.dma_start(wq_s[:], wq_r[:])
        xt_chunk = lambda tb: nc.sync.dma_start(
            xT_s[:, :, tb * 512 : (tb + 1) * 512],
            xT_r[:, :, tb * 512 : (tb + 1) * 512],
        )
        xt_chunk(0)
        nc.sync.dma_start(wk_s[:], wk_r[:])
        xt_chunk(1)
        xt_chunk(2)
        xt_chunk(3)
        nc.sync.dma_start(wv_s[:], wv_r[:])
        # consts staged through a DVE copy: consumers then depend on DVE
        # program order instead of a DMA semaphore (walrus 1-wait limit)
        nc.gpsimd.dma_start(mask_st[:], masks_d.ap())
        nc.gpsimd.dma_start(bq_st[:], bq_d.ap().rearrange("(o p) -> p o", p=128))
        nc.gpsimd.dma_start(bk_st[:], bk_d.ap().rearrange("(o p) -> p o", p=128))
        nc.gpsimd.dma_start(bv_row[:], bv_d.ap()[None, :])
        nc.vector.tensor_copy(mask_s[:], mask_st[:])
        nc.vector.tensor_copy(bq_s[:], bq_st[:])
        nc.vector.tensor_copy(bk_s[:], bk_st[:])
        nc.vector.tensor_copy(bv_row_bf[:], bv_row[:])

        def qk_unit(w_s, b_s, dst, mt, tb):
            def unit():
                ps = pps.tile([128, 512], F32, tag="proj_ps")
                for ct in range(CT):
                    nc.tensor.matmul(
                        ps[:],
                        w_s[:, ct, mt * 128 : (mt + 1) * 128],
                        xT_s[:, ct, tb * 512 : (tb + 1) * 512],
                        start=(ct == 0),
                        stop=(ct == CT - 1),
                    )
                nc.vector.tensor_scalar(
                    dst[:, mt, tb * 512 : (tb + 1) * 512],
                    ps[:],
                    b_s[:, mt : mt + 1],
                    None,
                    mybir.AluOpType.add,
                )
            return unit

        def v_unit(tt):
            def unit():
                ps = pps.tile([128, MH], F32, tag="v_ps")
                for ct in range(CT):
                    nc.tensor.matmul(
                        ps[:],
                        xT_s[:, ct, tt * 128 : (tt + 1) * 128],
                        wv_s[:, ct, :],
                        start=(ct == 0),
                        stop=False,
                    )
                # bias via K=1 ones-row accumulation
                nc.tensor.matmul(
                    ps[:], ones_bf128[:], bv_row_bf[:], start=False, stop=True
                )
                nc.vector.tensor_copy(
                    v_s[:, tt, :, 0:D], ps[:].rearrange("p (h d) -> p h d", h=LH)
                )
            return unit

        def qkt_units(p, ib):
            """QK^T + exp + diag-mask emission units for the 4 strips of
            i-block ib: one unit per (chunk, lh)."""
            units = []
            for jt in range(4 * ib, 4 * ib + 4):
                ia = 128 * jt
                w_all = T - ia
                off = 0
                while off < w_all:
                    cw = min(1024, w_all - off)
                    for lh in range(2):
                        def unit(p=p, jt=jt, ia=ia, off=off, cw=cw, lh=lh):
                            att_ps = aps.tile([128, 1024], F32, tag="att_ps")
                            prow = slice(64 * lh, 64 * lh + 64)
                            for s5 in range(0, cw, 512):
                                nn = min(512, cw - s5)
                                nc.tensor.matmul(
                                    att_ps[:, s5 : s5 + nn],
                                    kT_s[prow, p, jt * 128 : (jt + 1) * 128],
                                    qT_s[prow, p, ia + off + s5 : ia + off + s5 + nn],
                                    start=True,
                                    stop=True,
                                )
                            pcol = OFF[jt] + off
                            nc.scalar.activation(
                                PT[p][lh][:, pcol : pcol + cw],
                                att_ps[:, :cw],
                                mybir.ActivationFunctionType.Exp,
                                scale=float(SCALE),
                            )
                            if off == 0:
                                # diagonal 128x128 tile: zero j > i
                                nc.gpsimd.tensor_tensor(
                                    PT[p][lh][:, OFF[jt] : OFF[jt] + 128],
                                    PT[p][lh][:, OFF[jt] : OFF[jt] + 128],
                                    mask_s[:],
                                    mybir.AluOpType.mult,
                                )
                        units.append(unit)
                    off += cw
            return units

        def zip_emit(*streams):
            """Round-robin interleave emission units proportionally."""
            streams = [list(s) for s in streams if s]
            if not streams:
                return
            total = max(len(s) for s in streams)
            for k in range(total):
                for s in streams:
                    lo = k * len(s) // total
                    hi = (k + 1) * len(s) // total
                    for u in s[lo:hi]:
                        u()

        # emit just enough q/k for pair-0's first strips (q tb0-1, k tb0),
        # then overlap those strips' exp with the remaining projections
        qk_unit(wq_s, bq_s, qT_s, 0, 0)()
        qk_unit(wq_s, bq_s, qT_s, 0, 1)()
        qk_unit(wk_s, bk_s, kT_s, 0, 0)()
        rest = (
            [qk_unit(wq_s, bq_s, qT_s, 0, tb) for tb in (2, 3)]
            + [qk_unit(wk_s, bk_s, kT_s, 0, tb) for tb in (1, 2, 3)]
            + [qk_unit(wq_s, bq_s, qT_s, 1, tb) for tb in range(IB)]
            + [qk_unit(wk_s, bk_s, kT_s, 1, tb) for tb in range(IB)]
        )
        zip_emit(qkt_units(0, 0), rest)
        zip_emit(qkt_units(1, 0), [v_unit(tt) for tt in range(TT)])

        # projections done: free xT/weights SBUF and proj PSUM banks
        pps_cm.__exit__(None, None, None)
        pin_cm.__exit__(None, None, None)

        # ---- attention pipeline -----------------------------------------
        post = ctx.enter_context(tc.tile_pool(name="post", bufs=1))
        yps = ctx.enter_context(tc.tile_pool(name="yt_ps", bufs=2, space="PSUM"))
        aux = ctx.enter_context(tc.tile_pool(name="aux_ps", bufs=2, space="PSUM"))
        npool = ctx.enter_context(tc.tile_pool(name="norm", bufs=2))
        osb = ctx.enter_context(tc.tile_pool(name="out_sb", bufs=2))
        wp_s = post.tile([128, 2, C], BF16, tag="wp")
        yTn_s = post.tile([128, 2, T], BF16, tag="yTn")
        yTu = [
            npool.tile([64, 8, 512], BF16, tag=f"yTu{p}", name=f"yTu{p}", bufs=1)
            for p in range(2)
        ]
        nc.sync.dma_start(wp_s[:], wp_d.ap().rearrange("(o p) n -> p o n", p=128))
        out_r = out_d.ap().rearrange("(tt p) n -> tt p n", p=128)

        def pv_units(p, ib, yT_ps):
            units = []
            for jt in range(4 * ib + 4):
                for lh in range(2):
                    def unit(p=p, ib=ib, jt=jt, lh=lh):
                        ia = 128 * jt
                        c0 = max(512 * ib, ia)
                        nc.tensor.matmul(
                            yT_ps[lh][:, c0 - 512 * ib : 512],
                            v_s[:, jt, 2 * p + lh, :],
                            PT[p][lh][
                                :, OFF[jt] + c0 - ia : OFF[jt] + 512 * ib + 512 - ia
                            ],
                            start=(jt == 0),
                            stop=(jt == 4 * ib + 3),
                        )
                    units.append(unit)
            return units

        def norm(p, ib, yT_ps):
            # lazy per-ib normalization: 1/s off the staged denominator
            # row, K=1 matmul broadcast to 64 partitions, multiply
            for lh in range(2):
                r = ib * 2 + lh
                nc.vector.tensor_copy(yTu[p][:, r, :], yT_ps[lh][0:D, :])
                sst = npool.tile([1, 512], F32, tag="sst", name="sst")
                nc.vector.tensor_copy(sst[:], yT_ps[lh][D : D + 1, :])
                rsf = npool.tile([1, 512], F32, tag="rsf", name="rsf")
                # reciprocal_approx_fast misreads PSUM sources (HW-verified);
                # stage the row through SBUF first
                nc.vector.reciprocal_approx_fast(rsf[:], sst[:])
                rsb = npool.tile([1, 512], BF16, tag="rsb", name="rsb")
                nc.vector.tensor_copy(rsb[:], rsf[:])
                S_t = aux.tile([128, 512], F32, tag="aux", name="S_t")
                nc.tensor.matmul(
                    S_t[0:64, :], ones_bf[:], rsb[:], start=True, stop=True
                )
                nc.vector.tensor_tensor(
                    yTn_s[64 * lh : 64 * lh + 64, p, 512 * ib : 512 * ib + 512],
                    yTu[p][:, r, :],
                    S_t[0:64, :],
                    mybir.AluOpType.mult,
                )

        def out_units(ib):
            units = []
            for tt in range(4 * ib, 4 * ib + 4):
                for nb in range(2):
                    def unit(tt=tt, nb=nb):
                        o_ps = aux.tile([128, 512], F32, tag="aux", name="o_ps")
                        for pp in range(2):
                            nc.tensor.matmul(
                                o_ps[:],
                                yTn_s[:, pp, tt * 128 : (tt + 1) * 128],
                                wp_s[:, pp, nb * 512 : (nb + 1) * 512],
                                start=(pp == 0),
                                stop=(pp == 1),
                            )
                        ot = osb.tile([128, 512], BF16, tag="out_t")
                        nc.vector.tensor_copy(ot[:], o_ps[:])
                        nc.sync.dma_start(
                            out_r[tt, :, nb * 512 : (nb + 1) * 512], ot[:]
                        )
                    units.append(unit)
            return units

        pending_out = []
        for ib in range(IB):
            yT0 = [
                yps.tile([D + 1, 512], F32, tag="yT_ps", name=f"yT0_{lh}")
                for lh in range(2)
            ]
            half = len(pending_out) // 2
            zip_emit(
                pv_units(0, ib, yT0),
                qkt_units(0, ib + 1) if ib + 1 < IB else [],
                pending_out[:half],
            )
            norm(0, ib, yT0)
            yT1 = [
                yps.tile([D + 1, 512], F32, tag="yT_ps", name=f"yT1_{lh}")
                for lh in range(2)
            ]
            zip_emit(
                pv_units(1, ib, yT1),
                qkt_units(1, ib + 1) if ib + 1 < IB else [],
                pending_out[half:],
            )
            norm(1, ib, yT1)
            pending_out = out_units(ib)
        for u in pending_out:
            u()


_NC_CACHE = None


def get_nc() -> bass.Bass:
    global _NC_CACHE
    if _NC_CACHE is None:
        nc = bacc.Bacc()
        xT_d = nc.declare_dram_parameter("xT", [C, T], BF16, isOutput=False)
        wq_d = nc.declare_dram_parameter("wq", [C, MH], BF16, isOutput=False)
        wk_d = nc.declare_dram_parameter("wk", [C, MH], BF16, isOutput=False)
        wv_d = nc.declare_dram_parameter("wv", [C, MH], BF16, isOutput=False)
        wp_d = nc.declare_dram_parameter("wp", [MH, C], BF16, isOutput=False)
        bq_d = nc.declare_dram_parameter("bq", [MH], F32, isOutput=False)
        bk_d = nc.declare_dram_parameter("bk", [MH], F32, isOutput=False)
        bv_d = nc.declare_dram_parameter("bv", [MH], F32, isOutput=False)
        out_d = nc.declare_dram_parameter("out", [T, C], BF16, isOutput=True)
        masks_d = nc.inline_tensor(_causal_mask(), name="causal_mask")
        emit_kernel(
            nc, xT_d, wq_d, wk_d, wv_d, wp_d, bq_d, bk_d, bv_d, out_d, masks_d
        )
        nc.finalize()
        _NC_CACHE = nc
    return _NC_CACHE


def make_in_maps(x, Wq, bq, Wk, bk, Wv, bv, Wp, bp):
    in_maps = []
    for core in range(N_CORES):
        b, g = divmod(core, GROUPS)
        sl = slice(g * MH, (g + 1) * MH)
        in_maps.append(
            {
                "xT": np.ascontiguousarray(x[b].T).astype(NP_BF16),
                "wq": np.ascontiguousarray(Wq[:, sl]).astype(NP_BF16),
                "wk": np.ascontiguousarray(Wk[:, sl]).astype(NP_BF16),
                "wv": np.ascontiguousarray(Wv[:, sl]).astype(NP_BF16),
                "wp": np.ascontiguousarray(Wp[sl, :]).astype(NP_BF16),
                "bq": np.ascontiguousarray(bq[sl]).astype(np.float32),
                "bk": np.ascontiguousarray(bk[sl]).astype(np.float32),
                "bv": np.ascontiguousarray(bv[sl]).astype(np.float32),
            }
        )
    return in_maps


def kernel(x, Wq, bq, Wk, bk, Wv, bv, Wp, bp, _results_hook=None, _trace=False):
    x = np.asarray(x, dtype=np.float32)
    nc = get_nc()
    in_maps = make_in_maps(x, Wq, bq, Wk, bk, Wv, bv, Wp, bp)
    res = run_bass_kernel_spmd(
        nc, in_maps, core_ids=list(range(N_CORES)), trace=_trace
    )
    if _results_hook is not None:
        _results_hook(res)
    out = np.zeros((B, T, C), dtype=np.float32)
    for core in range(N_CORES):
        b = core // GROUPS
        out[b] += res.results[core]["out"].astype(np.float32)
    out += np.asarray(bp, dtype=np.float32)[None, None, :]
    return out


# revision 10
# speedup vs baseline: 1.2337x; 1.0110x over previous
"""Causal self-attention Trainium2 kernel (8 NeuronCores, SPMD).

Sharding: 8 cores = 2 batches x 4 head-groups (4 heads of 64 dims each).
Each core computes full-sequence attention for its 4 heads plus the
partial output projection for its 256 y-columns; the host sums the 4
partials per batch and adds the output bias.

Layout strategy (no on-device transposes anywhere):
  - host supplies x[b].T as xT [C, T] (bf16)
  - qT, kT produced in [m, t] layout (W stationary, xT moving)
  - v produced in natural [t, m] layout (xT stationary, Wv moving), with
    bias folded in as an extra K=1 ones-row matmul and a ones column per
    head (M=65) so the attention-value matmul also emits the softmax
    denominator row for free
  - attT[j, i] = sum_d kT[d,j] qT[d,i]  (kT stationary K=64; two heads
    run concurrently via row-tiled tile_position)
  - exp on ScalarE (fused 1/sqrt(64) scale); causal diagonal blocks
    masked multiplicatively on GpSimd; PT strips packed triangularly
    (strip jt keeps only columns >= 128*jt) so both head-pairs' strips
    fit in SBUF at once
  - software-pipelined emission: pair-1 q/k projections interleave with
    pair-0's first exp strips, v-projection with pair-1's; per i-block
    the PV accumulation interleaves with the next i-block's QK^T/exp
    and the previous i-block's output projection
  - normalization: denominator row staged to SBUF, reciprocal_approx_
    fast, K=1 ones-matmul broadcast, multiply
  - out[t, n] = yT.T @ Wp partial, bf16, DMA'd out; host sums in f32
"""

import sys

for _p in ("/opt/trn_rl_repo",):
    if _p not in sys.path:
        sys.path.insert(0, _p)

from contextlib import ExitStack

import ml_dtypes
import numpy as np

import concourse.bass as bass
import concourse.tile as tile
from concourse import bacc, mybir
from concourse.bass_utils import run_bass_kernel_spmd

BF16 = mybir.dt.bfloat16
F32 = mybir.dt.float32
NP_BF16 = ml_dtypes.bfloat16

B, T, C = 2, 2048, 1024
H, D = 16, 64
N_CORES = 8
GROUPS = 4          # head groups (cores per batch)
MH = C // GROUPS    # 256 columns per core (4 heads)
LH = MH // D        # 4 local heads
CT = C // 128       # 8 contraction tiles
TT = T // 128       # 16 sequence tiles of 128
IB = T // 512       # 4 i-blocks of 512
SCALE = 1.0 / np.sqrt(D)

# packed triangular PT layout: strip jt holds columns [128*jt, T)
OFF = []
_o = 0
for _jt in range(TT):
    OFF.append(_o)
    _o += T - 128 * _jt
PT_W = _o  # 17408


def _causal_mask() -> np.ndarray:
    """mask[j, i] = 1.0 if j <= i else 0 (bf16), [128, 128]."""
    j = np.arange(128)[:, None]
    i = np.arange(128)[None, :]
    return (j <= i).astype(NP_BF16)


def emit_kernel(nc, xT_d, wq_d, wk_d, wv_d, wp_d, bq_d, bk_d, bv_d, out_d, masks_d):
    with tile.TileContext(nc) as tc, ExitStack() as ctx:
        # ---- long-lived tiles -------------------------------------------
        keep = ctx.enter_context(tc.tile_pool(name="keep", bufs=1))
        qT_s = keep.tile([128, 2, T], BF16, tag="qT")
        kT_s = keep.tile([128, 2, T], BF16, tag="kT")
        v_s = keep.tile([128, TT, LH, D + 1], BF16, tag="v")
        mask_st = keep.tile([128, 128], BF16, tag="mask_st")
        mask_s = keep.tile([128, 128], BF16, tag="mask")
        bq_st = keep.tile([128, 2], F32, tag="bq_st")
        bq_s = keep.tile([128, 2], F32, tag="bq")
        bk_st = keep.tile([128, 2], F32, tag="bk_st")
        bk_s = keep.tile([128, 2], F32, tag="bk")
        bv_row = keep.tile([1, MH], F32, tag="bv_row")
        bv_row_bf = keep.tile([1, MH], BF16, tag="bv_row_bf")
        ones_bf128 = keep.tile([1, 128], BF16, tag="ones_bf128")
        ones_bf = keep.tile([1, 64], BF16, tag="ones_bf")
        act_scr = keep.tile([1, 128], F32, tag="act_scr")

        nc.vector.memset(ones_bf128[:], 1.0)
        nc.vector.memset(ones_bf[:], 1.0)
        nc.vector.memset(act_scr[:], 0.0)
        nc.vector.memset(v_s[:, :, :, D : D + 1], 1.0)
        # preload the exp table set while input DMAs are in flight
        nc.scalar.activation(
            act_scr[:], act_scr[:], mybir.ActivationFunctionType.Exp, scale=1.0
        )

        # ---- projections + pair-0/1 first strips, software-pipelined ----
        ptp = ctx.enter_context(tc.tile_pool(name="pt", bufs=1))
        aps = ctx.enter_context(tc.tile_pool(name="att_ps", bufs=2, space="PSUM"))
        pin_cm = tc.tile_pool(name="proj_in", bufs=1)
        pin = pin_cm.__enter__()
        pps_cm = tc.tile_pool(name="proj_ps", bufs=2, space="PSUM")
        pps = pps_cm.__enter__()
        PT = [
            [ptp.tile([128, PT_W], BF16, tag=f"PT{p}{lh}", name=f"PT{p}{lh}")
             for lh in range(2)]
            for p in range(2)
        ]

        xT_s = pin.tile([128, CT, T], BF16, tag="xT")
        wq_s = pin.tile([128, CT, MH], BF16, tag="wq")
        wk_s = pin.tile([128, CT, MH], BF16, tag="wk")
        wv_s = pin.tile([128, CT, MH], BF16, tag="wv")
        xT_r = xT_d.ap().rearrange("(o p) t -> p o t", p=128)
        wq_r = wq_d.ap().rearrange("(o p) m -> p o m", p=128)
        wk_r = wk_d.ap().rearrange("(o p) m -> p o m", p=128)
        wv_r = wv_d.ap().rearrange("(o p) m -> p o m", p=128)

        # warm the PE clock gate with dummy matmuls during the input DMAs
        for _ in range(32):
            wps = pps.tile([128, 512], F32, tag="proj_ps", name="warm_ps")
            nc.tensor.matmul(
                wps[:, 0:128], ones_bf128[:], ones_bf128[:], start=True, stop=True
            )

        # SP issues DMAs in program order: first-needed data first
        nc.sync.dma_start(wq_s[:], wq_r[:])
        xt_chunk = lambda tb: nc.sync.dma_start(
            xT_s[:, :, tb * 512 : (tb + 1) * 512],
            xT_r[:, :, tb * 512 : (tb + 1) * 512],
        )
        xt_chunk(0)
        nc.sync.dma_start(wk_s[:], wk_r[:])
        xt_chunk(1)
        xt_chunk(2)
        xt_chunk(3)
        nc.sync.dma_start(wv_s[:], wv_r[:])
        # consts staged through a DVE copy: consumers then depend on DVE
        # program order instead of a DMA semaphore (walrus 1-wait limit)
        nc.gpsimd.dma_start(mask_st[:], masks_d.ap())
        nc.gpsimd.dma_start(bq_st[:], bq_d.ap().rearrange("(o p) -> p o", p=128))
        nc.gpsimd.dma_start(bk_st[:], bk_d.ap().rearrange("(o p) -> p o", p=128))
        nc.gpsimd.dma_start(bv_row[:], bv_d.ap()[None, :])
        nc.vector.tensor_copy(mask_s[:], mask_st[:])
        nc.vector.tensor_copy(bq_s[:], bq_st[:])
        nc.vector.tensor_copy(bk_s[:], bk_st[:])
        nc.vector.tensor_copy(bv_row_bf[:], bv_row[:])

        def qk_unit(w_s, b_s, dst, mt, tb):
            def unit():
                ps = pps.tile([128, 512], F32, tag="proj_ps")
                for ct in range(CT):
                    nc.tensor.matmul(
                        ps[:],
                        w_s[:, ct, mt * 128 : (mt + 1) * 128],
                        xT_s[:, ct, tb * 512 : (tb + 1) * 512],
                        start=(ct == 0),
                        stop=(ct == CT - 1),
                    )
                nc.vector.tensor_scalar(
                    dst[:, mt, tb * 512 : (tb + 1) * 512],
                    ps[:],
                    b_s[:, mt : mt + 1],
                    None,
                    mybir.AluOpType.add,
                )
            return unit

        def v_unit(tt):
            def unit():
                ps = pps.tile([128, MH], F32, tag="v_ps")
                for ct in range(CT):
                    nc.tensor.matmul(
                        ps[:],
                        xT_s[:, ct, tt * 128 : (tt + 1) * 128],
                        wv_s[:, ct, :],
                        start=(ct == 0),
                        stop=False,
                    )
                # bias via K=1 ones-row accumulation
                nc.tensor.matmul(
                    ps[:], ones_bf128[:], bv_row_bf[:], start=False, stop=True
                )
                nc.vector.tensor_copy(
                    v_s[:, tt, :, 0:D], ps[:].rearrange("p (h d) -> p h d", h=LH)
                )
            return unit

        def qkt_units(p, ib):
            """QK^T + exp + diag-mask emission units for the 4 strips of
            i-block ib: one unit per (chunk, lh)."""
            units = []
            for jt in range(4 * ib, 4 * ib + 4):
                ia = 128 * jt
                w_all = T - ia
                off = 0
                while off < w_all:
                    cw = min(1024, w_all - off)
                    for lh in range(2):
                        def unit(p=p, jt=jt, ia=ia, off=off, cw=cw, lh=lh):
                            att_ps = aps.tile([128, 1024], F32, tag="att_ps")
                            prow = slice(64 * lh, 64 * lh + 64)
                            for s5 in range(0, cw, 512):
                                nn = min(512, cw - s5)
                                nc.tensor.matmul(
                                    att_ps[:, s5 : s5 + nn],
                                    kT_s[prow, p, jt * 128 : (jt + 1) * 128],
                                    qT_s[prow, p, ia + off + s5 : ia + off + s5 + nn],
                                    start=True,
                                    stop=True,
                                )
                            pcol = OFF[jt] + off
                            nc.scalar.activation(
                                PT[p][lh][:, pcol : pcol + cw],
                                att_ps[:, :cw],
                                mybir.ActivationFunctionType.Exp,
                                scale=float(SCALE),
                            )
                            if off == 0:
                                # diagonal 128x128 tile: zero j > i
                                nc.gpsimd.tensor_tensor(
                                    PT[p][lh][:, OFF[jt] : OFF[jt] + 128],
                                    PT[p][lh][:, OFF[jt] : OFF[jt] + 128],
                                    mask_s[:],
                                    mybir.AluOpType.mult,
                                )
                        units.append(unit)
                    off += cw
            return units

        def zip_emit(*streams):
            """Round-robin interleave emission units proportionally."""
            streams = [list(s) for s in streams if s]
            if not streams:
                return
            total = max(len(s) for s in streams)
            for k in range(total):
                for s in streams:
                    lo = k * len(s) // total
                    hi = (k + 1) * len(s) // total
                    for u in s[lo:hi]:
                        u()

        # pair-0 q/k, then pair-0 strips overlapped with pair-1 q/k, then
        # pair-1 strips overlapped with the v projection
        for tb in range(IB):
            qk_unit(wq_s, bq_s, qT_s, 0, tb)()
        for tb in range(IB):
            qk_unit(wk_s, bk_s, kT_s, 0, tb)()
        zip_emit(
            qkt_units(0, 0),
            [qk_unit(wq_s, bq_s, qT_s, 1, tb) for tb in range(IB)]
            + [qk_unit(wk_s, bk_s, kT_s, 1, tb) for tb in range(IB)],
        )
        zip_emit(qkt_units(1, 0), [v_unit(tt) for tt in range(TT)])

        # projections done: free xT/weights SBUF and proj PSUM banks
        pps_cm.__exit__(None, None, None)
        pin_cm.__exit__(None, None, None)

        # ---- attention pipeline -----------------------------------------
        post = ctx.enter_context(tc.tile_pool(name="post", bufs=1))
        yps = ctx.enter_context(tc.tile_pool(name="yt_ps", bufs=2, space="PSUM"))
        aux = ctx.enter_context(tc.tile_pool(name="aux_ps", bufs=2, space="PSUM"))
        npool = ctx.enter_context(tc.tile_pool(name="norm", bufs=2))
        osb = ctx.enter_context(tc.tile_pool(name="out_sb", bufs=2))
        wp_s = post.tile([128, 2, C], BF16, tag="wp")
        yTn_s = post.tile([128, 2, T], BF16, tag="yTn")
        yTu = [
            npool.tile([64, 8, 512], BF16, tag=f"yTu{p}", name=f"yTu{p}", bufs=1)
            for p in range(2)
        ]
        nc.sync.dma_start(wp_s[:], wp_d.ap().rearrange("(o p) n -> p o n", p=128))
        out_r = out_d.ap().rearrange("(tt p) n -> tt p n", p=128)

        def pv_units(p, ib, yT_ps):
            units = []
            for jt in range(4 * ib + 4):
                for lh in range(2):
                    def unit(p=p, ib=ib, jt=jt, lh=lh):
                        ia = 128 * jt
                        c0 = max(512 * ib, ia)
                        nc.tensor.matmul(
                            yT_ps[lh][:, c0 - 512 * ib : 512],
                            v_s[:, jt, 2 * p + lh, :],
                            PT[p][lh][
                                :, OFF[jt] + c0 - ia : OFF[jt] + 512 * ib + 512 - ia
                            ],
                            start=(jt == 0),
                            stop=(jt == 4 * ib + 3),
                        )
                    units.append(unit)
            return units

        def norm(p, ib, yT_ps):
            # lazy per-ib normalization: 1/s off the staged denominator
            # row, K=1 matmul broadcast to 64 partitions, multiply
            for lh in range(2):
                r = ib * 2 + lh
                nc.vector.tensor_copy(yTu[p][:, r, :], yT_ps[lh][0:D, :])
                sst = npool.tile([1, 512], F32, tag="sst", name="sst")
                nc.vector.tensor_copy(sst[:], yT_ps[lh][D : D + 1, :])
                rsf = npool.tile([1, 512], F32, tag="rsf", name="rsf")
                # reciprocal_approx_fast misreads PSUM sources (HW-verified);
                # stage the row through SBUF first
                nc.vector.reciprocal_approx_fast(rsf[:], sst[:])
                rsb = npool.tile([1, 512], BF16, tag="rsb", name="rsb")
                nc.vector.tensor_copy(rsb[:], rsf[:])
                S_t = aux.tile([128, 512], F32, tag="aux", name="S_t")
                nc.tensor.matmul(
                    S_t[0:64, :], ones_bf[:], rsb[:], start=True, stop=True
                )
                nc.vector.tensor_tensor(
                    yTn_s[64 * lh : 64 * lh + 64, p, 512 * ib : 512 * ib + 512],
                    yTu[p][:, r, :],
                    S_t[0:64, :],
                    mybir.AluOpType.mult,
                )

        def out_units(ib):
            units = []
            for tt in range(4 * ib, 4 * ib + 4):
                for nb in range(2):
                    def unit(tt=tt, nb=nb):
                        o_ps = aux.tile([128, 512], F32, tag="aux", name="o_ps")
                        for pp in range(2):
                            nc.tensor.matmul(
                                o_ps[:],
                                yTn_s[:, pp, tt * 128 : (tt + 1) * 128],
                                wp_s[:, pp, nb * 512 : (nb + 1) * 512],
                                start=(pp == 0),
                                stop=(pp == 1),
                            )
                        ot = osb.tile([128, 512], BF16, tag="out_t")
                        nc.vector.tensor_copy(ot[:], o_ps[:])
                        nc.sync.dma_start(
                            out_r[tt, :, nb * 512 : (nb + 1) * 512], ot[:]
                        )
                    units.append(unit)
            return units

        pending_out = []
        for ib in range(IB):
            yT0 = [
                yps.tile([D + 1, 512], F32, tag="yT_ps", name=f"yT0_{lh}")
                for lh in range(2)
            ]
            half = len(pending_out) // 2
            zip_emit(
                pv_units(0, ib, yT0),
                qkt_units(0, ib + 1) if ib + 1 < IB else [],
                pending_out[:half],
            )
            norm(0, ib, yT0)
            yT1 = [
                yps.tile([D + 1, 512], F32, tag="yT_ps", name=f"yT1_{lh}")
                for lh in range(2)
            ]
            zip_emit(
                pv_units(1, ib, yT1),
                qkt_units(1, ib + 1) if ib + 1 < IB else [],
                pending_out[half:],
            )
            norm(1, ib, yT1)
            pending_out = out_units(ib)
        for u in pending_out:
            u()


_NC_CACHE = None


def get_nc() -> bass.Bass:
    global _NC_CACHE
    if _NC_CACHE is None:
        nc = bacc.Bacc()
        xT_d = nc.declare_dram_parameter("xT", [C, T], BF16, isOutput=False)
        wq_d = nc.declare_dram_parameter("wq", [C, MH], BF16, isOutput=False)
        wk_d = nc.declare_dram_parameter("wk", [C, MH], BF16, isOutput=False)
        wv_d = nc.declare_dram_parameter("wv", [C, MH], BF16, isOutput=False)
        wp_d = nc.declare_dram_parameter("wp", [MH, C], BF16, isOutput=False)
        bq_d = nc.declare_dram_parameter("bq", [MH], F32, isOutput=False)
        bk_d = nc.declare_dram_parameter("bk", [MH], F32, isOutput=False)
        bv_d = nc.declare_dram_parameter("bv", [MH], F32, isOutput=False)
        out_d = nc.declare_dram_parameter("out", [T, C], BF16, isOutput=True)
        masks_d = nc.inline_tensor(_causal_mask(), name="causal_mask")
        emit_kernel(
            nc, xT_d, wq_d, wk_d, wv_d, wp_d, bq_d, bk_d, bv_d, out_d, masks_d
        )
        nc.finalize()
        _NC_CACHE = nc
    return _NC_CACHE


def make_in_maps(x, Wq, bq, Wk, bk, Wv, bv, Wp, bp):
    in_maps = []
    for core in range(N_CORES):
        b, g = divmod(core, GROUPS)
        sl = slice(g * MH, (g + 1) * MH)
        in_maps.append(
            {
                "xT": np.ascontiguousarray(x[b].T).astype(NP_BF16),
                "wq": np.ascontiguousarray(Wq[:, sl]).astype(NP_BF16),
                "wk": np.ascontiguousarray(Wk[:, sl]).astype(NP_BF16),
                "wv": np.ascontiguousarray(Wv[:, sl]).astype(NP_BF16),
                "wp": np.ascontiguousarray(Wp[sl, :]).astype(NP_BF16),
                "bq": np.ascontiguousarray(bq[sl]).astype(np.float32),
                "bk": np.ascontiguousarray(bk[sl]).astype(np.float32),
                "bv": np.ascontiguousarray(bv[sl]).astype(np.float32),
            }
        )
    return in_maps


def kernel(x, Wq, bq, Wk, bk, Wv, bv, Wp, bp, _results_hook=None, _trace=False):
    x = np.asarray(x, dtype=np.float32)
    nc = get_nc()
    in_maps = make_in_maps(x, Wq, bq, Wk, bk, Wv, bv, Wp, bp)
    res = run_bass_kernel_spmd(
        nc, in_maps, core_ids=list(range(N_CORES)), trace=_trace
    )
    if _results_hook is not None:
        _results_hook(res)
    out = np.zeros((B, T, C), dtype=np.float32)
    for core in range(N_CORES):
        b = core // GROUPS
        out[b] += res.results[core]["out"].astype(np.float32)
    out += np.asarray(bp, dtype=np.float32)[None, None, :]
    return out
